# revision 1
# baseline (speedup 1.0000x reference)
"""KoLeo loss kernel for 8 Trainium2 NeuronCores.

Reference computation (B=16384, D=1024):
    xn  = x / max(||x||_2, 1e-12)          # row L2-normalize
    sim = xn @ xn.T                        # B x B cosine similarity
    max_sim[i] = max_{j != i} sim[i, j]    # nearest neighbor (excl. self)
    out = -mean(log(sqrt(2 - 2*max_sim + 1e-8)))

Sharding: rows of x are split across 8 cores (2048 rows each). Each core
computes its 2048 x 16384 slab of the similarity matrix against all of x
(streamed), takes the row max, and returns the per-row maxima. The cheap
nonlinear epilogue (sqrt/log/mean over 16384 scalars) runs on the host in
float64.

Implementation notes:
  - The host pre-normalizes rows (float64) and pre-transposes to x.T
    [D, B] in fp8e4m3 (or bf16), so the device does no transposes and no
    normalization: the kernel is a pure matmul + diagonal fix + row-max.
    fp8 uses DoubleRow perf mode (256-deep contraction per matmul, 2x).
  - Per-core input is x.T *rotated* so each core's own rows are columns
    0..2047. This makes the self-similarity diagonal land at a fixed,
    core-independent position, keeping the program identical across cores
    (pure SPMD): the slab's diagonal entries sit exactly on the diagonal
    of the leading 2048x2048 block. We subtract 2 there (via a -2*I
    constant) before the row max so the self-match (cos=1) never wins.
  - The leading 4 j-blocks of the rhs stream are slices of the resident
    lhsT tile, so only 28/32 column blocks are DMA'd.
"""

import sys

if "/opt/trn_rl_repo" not in sys.path:
    sys.path.insert(0, "/opt/trn_rl_repo")

import numpy as np
import ml_dtypes

import concourse.bass as bass  # noqa: F401  (import keeps bass registered)
import concourse.mybir as mybir
import concourse.tile as tile
from concourse import bacc
from concourse.bass_utils import run_bass_kernel_spmd

P = 128          # SBUF partitions
NBLK = 512       # similarity column block width (= one PSUM bank of f32)
EPS = 1e-8

B = 16384        # rows of x
D = 1024         # feature dim
N_CORES = 8

# Compute mode: "bf16" (1 cycle/row) or "fp8dr" (fp8e4m3 + DoubleRow,
# 0.5 cycles/row). fp8 inputs are pre-scaled by FP8_SCALE so the unit-norm
# components (sigma ~ 1/32) sit in e4m3's normal range; similarities then
# come out scaled by FP8_SCALE**2, undone on the host.
import os as _os

# fp8dr measured 461,913 ns / rel err 1.6e-3 on HW; bf16 measured
# 901,380 ns / rel err 2.7e-6. Both pass; fp8dr is ~1.95x faster and
# keeps >10x margin to the 2e-2 family accuracy gate.
MODE = _os.environ.get("KOLEO_MODE", "fp8dr")
FP8_SCALE = 8.0


def build_nc(b=B, d=D, n_cores=N_CORES, mode=MODE):
    """Build the per-core SPMD Bass program.

    Inputs :  xt     [d, b]  bf16/fp8e4m3 — rotated, normalized x.T
              negeye [P, P]  f32 — the constant -2*scale^2 * I
    Output :  out    [P, b//n_cores//P] f32 — out[p, m] = scale^2 *
              max_{j != i} sim[i, j] for local row i = m*P + p
    """
    bl = b // n_cores          # local rows per core
    kch = d // P               # contraction chunks
    mch = bl // P              # output row chunks
    nb = b // NBLK             # column blocks
    diag_nb = bl // NBLK       # leading blocks that overlap the diagonal
    assert bl % NBLK == 0 and d % P == 0 and b % NBLK == 0

    if mode == "bf16":
        in_dt = mybir.dt.bfloat16
        kstep = 1                      # K chunks of 128 per matmul
        perf_mode = None
    else:
        in_dt = mybir.dt.float8e4      # e4m3
        kstep = 2                      # DoubleRow: K chunks of 256
        perf_mode = mybir.MatmulPerfMode.DoubleRow
        assert kch % 2 == 0

    nc = bacc.Bacc("TRN2", target_bir_lowering=False, debug=False,
                   num_devices=n_cores)
    xt = nc.dram_tensor("xt", [d, b], in_dt, kind="ExternalInput")
    negeye = nc.dram_tensor("negeye", [P, P], mybir.dt.float32,
                            kind="ExternalInput")
    out = nc.dram_tensor("out", [P, mch], mybir.dt.float32,
                         kind="ExternalOutput")
    xt_ap = xt[:]
    f32 = mybir.dt.float32

    with tile.TileContext(nc) as tc:
        with (
            tc.tile_pool(name="lhs", bufs=1) as lhs_pool,
            tc.tile_pool(name="rhs", bufs=4) as rhs_pool,
            tc.tile_pool(name="psum", bufs=8, space="PSUM") as psum_pool,
            tc.tile_pool(name="stats", bufs=1) as stats_pool,
        ):
            # Alternate DMA issue between two engines so chunk transfers
            # land on different queues and run in parallel.
            dma_eng = [nc.sync, nc.gpsimd]

            # Resident lhsT: this core's rows, K-on-partitions, one tile
            # per kstep group so the first matmul starts after the first
            # group's DMA instead of the whole slab.
            lhs_tiles = [
                lhs_pool.tile([P, kstep, bl], in_dt, name=f"lhs{g}",
                              tag=f"lhs{g}")
                for g in range(kch // kstep)
            ]
            for k in range(kch):
                g, o = divmod(k, kstep)
                dma_eng[k % 2].dma_start(lhs_tiles[g][:, o, :],
                                         xt_ap[k * P:(k + 1) * P, 0:bl])
            eye = stats_pool.tile([P, P], f32, name="eye")
            nc.gpsimd.dma_start(eye[:], negeye[:])

            maxtiles = [
                stats_pool.tile([P, nb], f32, name=f"maxt{m}", tag=f"maxt{m}")
                for m in range(mch)
            ]
            rowmax = stats_pool.tile([P, mch], f32, name="rowmax")

            for jb in range(nb):
                cols = slice(jb * NBLK, (jb + 1) * NBLK)
                if jb < diag_nb:
                    # rhs block is part of the resident lhs tiles
                    rt = None
                else:
                    rt = rhs_pool.tile([P, kch, NBLK], in_dt, name="rt", tag="rt")
                    for k in range(kch):
                        dma_eng[k % 2].dma_start(rt[:, k, :],
                                                 xt_ap[k * P:(k + 1) * P, cols])
                for m in range(mch):
                    ps = psum_pool.tile([P, NBLK], f32, name="ps", tag="ps")
                    for g in range(kch // kstep):
                        k = g * kstep
                        if kstep == 1:
                            rhs = (lhs_tiles[g][:, 0, cols] if rt is None
                                   else rt[:, k, :])
                            lhsT = lhs_tiles[g][:, 0, m * P:(m + 1) * P]
                        else:
                            rhs = (lhs_tiles[g][:, :, cols] if rt is None
                                   else rt[:, k:k + kstep, :])
                            lhsT = lhs_tiles[g][:, :, m * P:(m + 1) * P]
                        nc.tensor.matmul(
                            ps[:],
                            lhsT,
                            rhs,
                            start=(g == 0),
                            stop=(k + kstep == kch),
                            perf_mode=perf_mode,
                        )
                    if jb == (m * P) // NBLK:
                        # self-similarity lives at ps[p, off + p]: add -2*I
                        off = (m * P) % NBLK
                        nc.vector.tensor_add(
                            out=ps[:, off:off + P],
                            in0=ps[:, off:off + P],
                            in1=eye[:],
                        )
                    nc.vector.reduce_max(
                        out=maxtiles[m][:, jb:jb + 1],
                        in_=ps[:],
                        axis=mybir.AxisListType.X,
                        op=mybir.AluOpType.max,
                    )

            for m in range(mch):
                nc.vector.reduce_max(
                    out=rowmax[:, m:m + 1],
                    in_=maxtiles[m][:],
                    axis=mybir.AxisListType.X,
                    op=mybir.AluOpType.max,
                )
            nc.sync.dma_start(out[:], rowmax[:])

    nc.compile()
    return nc


def prepare_inputs(x, b=B, d=D, n_cores=N_CORES, mode=MODE):
    """Host prep: normalize (f64), transpose, cast, per-core rotate."""
    bl = b // n_cores
    xd = np.asarray(x, dtype=np.float64)
    norms = np.sqrt(np.einsum("ij,ij->i", xd, xd))
    np.maximum(norms, 1e-12, out=norms)
    xn = xd / norms[:, None]
    if mode == "bf16":
        scale = 1.0
        xnt = np.ascontiguousarray(xn.T).astype(ml_dtypes.bfloat16)
    else:
        scale = FP8_SCALE
        xnt = np.ascontiguousarray(xn.T * scale).astype(ml_dtypes.float8_e4m3)
    negeye = np.ascontiguousarray(
        (-2.0 * scale * scale) * np.eye(P, dtype=np.float32))
    in_maps = []
    for c in range(n_cores):
        s = c * bl
        rot = np.concatenate([xnt[:, s:], xnt[:, :s]], axis=1) if s else xnt
        in_maps.append({"xt": np.ascontiguousarray(rot), "negeye": negeye})
    return in_maps


def postprocess(results, b=B, n_cores=N_CORES, mode=MODE):
    """Stitch per-core row-max outputs and apply the scalar epilogue."""
    bl = b // n_cores
    inv = 1.0 if mode == "bf16" else 1.0 / (FP8_SCALE * FP8_SCALE)
    maxsim = np.empty(b, dtype=np.float64)
    for c in range(n_cores):
        o = np.asarray(results[c]["out"], dtype=np.float64)  # [P, mch]
        maxsim[c * bl:(c + 1) * bl] = o.T.reshape(-1) * inv  # i = m*P + p
    d2 = 2.0 - 2.0 * maxsim + EPS
    loss = -0.5 * np.mean(np.log(d2))
    return np.array(loss, dtype=np.float32)


_NC_CACHE = {}


def _get_nc():
    key = (B, D, N_CORES, MODE)
    if key not in _NC_CACHE:
        _NC_CACHE[key] = build_nc(*key)
    return _NC_CACHE[key]


def kernel(x, **_ignored):
    nc = _get_nc()
    in_maps = prepare_inputs(x)
    last_exc = None
    for _attempt in range(3):
        try:
            res = run_bass_kernel_spmd(nc, in_maps,
                                       core_ids=list(range(N_CORES)))
            return postprocess(res.results)
        except Exception as exc:  # transient NRT/tunnel hiccups
            last_exc = exc
    raise last_exc


if __name__ == "__main__":
    x = np.random.default_rng(0).standard_normal((B, D), dtype=np.float32)
    print(kernel(x))



# revision 8
# speedup vs baseline: 1.3596x; 1.3596x over previous
"""KoLeo loss kernel for 8 Trainium2 NeuronCores — symmetric-half version.

Reference computation (B=16384, D=1024):
    xn  = x / max(||x||_2, 1e-12)          # row L2-normalize
    sim = xn @ xn.T                        # B x B cosine similarity
    max_sim[i] = max_{j != i} sim[i, j]    # nearest neighbor (excl. self)
    out = -mean(log(sqrt(2 - 2*max_sim + 1e-8)))

Sharding + symmetry: rows of x are split across 8 cores (2048 rows each).
sim is symmetric, so each computed entry sim[i, j] can serve both row i's
max (row-max over the streamed block) and row j's max (column-max,
accumulated across row chunks).  Each 128-row chunk m of a core computes
only a 17x512 = 8704-column window starting at its own diagonal (rotated
frame), instead of the full 16384 columns.  Windows of 8704 columns
guarantee every unordered pair {a, b} is covered by at least one of the
two owning chunks: the pair is missed by chunk(a) iff
(a%128 + delta) mod B in [8704, B) and by chunk(b) iff delta in
(b%128, b%128 + B - 8704]; the intersection is empty because
2*8704 > B + 254.  This cuts the matmul work to 17/32 = 0.53x.

Engine pipeline per 512-column psum block:
    PE   : 4 fp8-DoubleRow matmuls  -> ps [128, 512] f32 (PSUM) (853 ns)
    Act  : copy ps -> blk [128, 512] fp16 (SBUF)                (~570 ns)
    DVE  : (t==0: blk[:, :128] += -2*scale^2*I)
           row-max of blk -> rowacc  (tensor_tensor_reduce with
           in0 = in1 = blk: identity elementwise + max reduction)
           acc[:, s:s+512] = max(blk, acc)  (column-max, tensor_max)
The fp16 bounce copy gives the DVE 2-byte SBUF operands (2x tier for
TensorTensor-class ops; GPSIMD cannot run them and cannot read PSUM).
Row-max reduces only the fp16 block (never the cross-chunk
accumulator), so it is exact: the accumulator holds other rows' maxima
and must not leak into row-max.

Host finishes: per-column partition max of acc, scatter-max into the
global per-row max (the rotation makes that two slice maxes per core),
then the scalar log/sqrt/mean epilogue in float64.

Implementation notes:
  - Host pre-normalizes rows (f64) and pre-transposes to x.T in fp8e4m3
    scaled by 8 (DoubleRow perf mode, 2x matmul throughput; similarities
    come out scaled by 64, undone on the host).
  - Per-core input is x.T rotated so the core's own rows are columns
    0..2047; chunk m's window starts at column 128*m, so the
    self-similarity diagonal sits at window offset [0, 128) for every
    chunk — one -2*scale^2*I add per chunk kills the self-match.
  - Blocks are processed sorted by window-start column so the resident
    rhs DMA streams strictly left-to-right and finalized slices of the
    column-max accumulator DMA out while compute continues (no tail).
  - One resident SBUF tensor R [128, 8, 10624] (fp8) serves as both
    matmul weights (own rows = cols 0..2047) and moving data.
"""

import sys

if "/opt/trn_rl_repo" not in sys.path:
    sys.path.insert(0, "/opt/trn_rl_repo")

import numpy as np
import ml_dtypes

import concourse.bass as bass  # noqa: F401  (import keeps bass registered)
import concourse.mybir as mybir
import concourse.tile as tile
from concourse import bacc
from concourse.bass_utils import run_bass_kernel_spmd

P = 128          # SBUF partitions
NBLK = 512       # similarity column block width (= one PSUM bank of f32)
EPS = 1e-8

B = 16384        # rows of x
D = 1024         # feature dim
N_CORES = 8
BL = B // N_CORES          # local rows per core (2048)
MCH = BL // P              # row chunks per core (16)
W = 17                     # 512-col blocks per chunk window (8704 cols)
CW = (MCH - 1) * P + W * NBLK   # resident rotated columns (10624)
KCH = D // P               # contraction chunks (8)
KSTEP = 2                  # fp8 DoubleRow: K chunks of 256

FP8_SCALE = 8.0
NEG_INIT = -60000.0        # fp16-representable, far below any -128..128 sim

import os as _os

ROWRED = _os.environ.get("KOLEO_ROWRED", "ttr")   # "ttr" | "reduce"
MEMSET_ENG = _os.environ.get("KOLEO_MEMSET", "gpsimd")  # "gpsimd" | "vector"


def build_nc(n_cores=N_CORES):
    """Build the per-core SPMD Bass program.

    Inputs :  xt     [D, CW] fp8e4m3 — rotated, normalized, scaled x.T
              negeye [P, P]  f16 — the constant -2*scale^2 * I
    Outputs:  rowmax [P, MCH] f32 — rowmax[p, m] = scale^2 *
              max_{j in window} sim[128m+p, j] (excl. self)
              colacc [P, CW] f16 — colacc[p, c] = scale^2 *
              max over chunks m (with c in window m) of sim[128m+p, c]
    """
    in_dt = mybir.dt.float8e4
    f32 = mybir.dt.float32
    f16 = mybir.dt.float16
    perf_mode = mybir.MatmulPerfMode.DoubleRow

    nc = bacc.Bacc("TRN2", target_bir_lowering=False, debug=False,
                   num_devices=n_cores)
    xt = nc.dram_tensor("xt", [D, CW], in_dt, kind="ExternalInput")
    negeye = nc.dram_tensor("negeye", [P, P], f16, kind="ExternalInput")
    rowmax_out = nc.dram_tensor("rowmax", [P, MCH], f32,
                                kind="ExternalOutput")
    colacc_out = nc.dram_tensor("colacc", [P, CW], f16,
                                kind="ExternalOutput")
    xt_ap = xt[:]

    with tile.TileContext(nc) as tc:
        with (
            tc.tile_pool(name="data", bufs=1) as data_pool,
            tc.tile_pool(name="blk", bufs=8) as blk_pool,
            tc.tile_pool(name="psum", bufs=8, space="PSUM") as psum_pool,
            tc.tile_pool(name="stats", bufs=1) as stats_pool,
        ):
            R = data_pool.tile([P, KCH, CW], in_dt, name="R")
            acc = data_pool.tile([P, CW], f16, name="acc")
            eye = stats_pool.tile([P, P], f16, name="eye")
            rowaccs = [
                stats_pool.tile([P, W], f32, name=f"rowacc{m}",
                                tag=f"rowacc{m}")
                for m in range(MCH)
            ]
            rowmax = stats_pool.tile([P, MCH], f32, name="rowmax")

            # col-max accumulator starts far below any similarity; both
            # engines are idle until the first block lands.
            memset_eng = nc.gpsimd if MEMSET_ENG == "gpsimd" else nc.vector
            memset_eng.memset(acc[:], NEG_INIT)
            nc.sync.dma_start(eye[:], negeye[:])

            # Stream the rotated slab in column-piece order so the PE
            # (which walks blocks by ascending window start) never waits
            # long.  sync + scalar are the two HWDGE queues; gpsimd
            # (software DGE) is kept free for the column-max stream.
            dma_eng = [nc.sync, nc.scalar]
            n_pieces = (CW + NBLK - 1) // NBLK
            di = 0
            for j in range(n_pieces):
                c0, c1 = j * NBLK, min(CW, (j + 1) * NBLK)
                for k in range(KCH):
                    dma_eng[di % 2].dma_start(
                        R[:, k, c0:c1], xt_ap[k * P:(k + 1) * P, c0:c1])
                    di += 1

            # All (chunk, block) pairs sorted by window-start column.
            blocks = sorted(
                (P * m + NBLK * t, m, t)
                for m in range(MCH) for t in range(W)
            )
            dma_ptr = 0
            for idx, (start, m, t) in enumerate(blocks):
                ps = psum_pool.tile([P, NBLK], f32, name="ps", tag="ps")
                for g in range(KCH // KSTEP):
                    k = g * KSTEP
                    lhsT = R[:, k:k + KSTEP, m * P:(m + 1) * P]
                    rhs = R[:, k:k + KSTEP, start:start + NBLK]
                    nc.tensor.matmul(
                        ps[:], lhsT, rhs,
                        start=(g == 0),
                        stop=(k + KSTEP == KCH),
                        perf_mode=perf_mode,
                    )
                # bounce PSUM f32 -> SBUF fp16 on the Act engine
                blk = blk_pool.tile([P, NBLK], f16, name="blk", tag="blk")
                nc.scalar.copy(blk[:], ps[:])
                if t == 0:
                    # self-similarity lives at blk[p, p]: add -2*scale^2*I
                    nc.vector.tensor_add(
                        out=blk[:, 0:P], in0=blk[:, 0:P], in1=eye[:])
                # exact row-max of this block only (never the accumulator).
                # tensor_tensor_reduce with in0 == in1 == out is an identity
                # elementwise pass (max(x, x) = x, rewriting blk with its
                # own values) whose reduction stage yields the row max —
                # the TensorTensor opcode class runs 2x on 2-byte SBUF
                # operands, unlike TensorReduce which is capped at 1x.
                if ROWRED == "ttr":
                    nc.vector.tensor_tensor_reduce(
                        out=blk[:],
                        in0=blk[:],
                        in1=blk[:],
                        scale=1.0,
                        scalar=NEG_INIT,
                        op0=mybir.AluOpType.max,
                        op1=mybir.AluOpType.max,
                        accum_out=rowaccs[m][:, t:t + 1],
                    )
                else:
                    nc.vector.reduce_max(
                        out=rowaccs[m][:, t:t + 1],
                        in_=blk[:],
                        axis=mybir.AxisListType.X,
                        op=mybir.AluOpType.max,
                    )
                # column-max accumulate (fp16 SBUF -> 2x DVE tier)
                nc.vector.tensor_max(
                    out=acc[:, start:start + NBLK],
                    in0=blk[:],
                    in1=acc[:, start:start + NBLK],
                )
                # Everything left of the next block's start is final —
                # stream it out while compute continues.
                next_start = CW if idx == len(blocks) - 1 else blocks[idx + 1][0]
                if next_start - dma_ptr >= 2048 or idx == len(blocks) - 1:
                    nc.sync.dma_start(
                        colacc_out[:, dma_ptr:next_start],
                        acc[:, dma_ptr:next_start])
                    dma_ptr = next_start

            for m in range(MCH):
                nc.vector.reduce_max(
                    out=rowmax[:, m:m + 1],
                    in_=rowaccs[m][:],
                    axis=mybir.AxisListType.X,
                    op=mybir.AluOpType.max,
                )
            nc.sync.dma_start(rowmax_out[:], rowmax[:])

    nc.compile()
    return nc


def prepare_inputs(x, n_cores=N_CORES):
    """Host prep: normalize (f64), transpose, scale+cast fp8, rotate."""
    xd = np.asarray(x, dtype=np.float64)
    norms = np.sqrt(np.einsum("ij,ij->i", xd, xd))
    np.maximum(norms, 1e-12, out=norms)
    xn = xd / norms[:, None]
    xnt = np.ascontiguousarray(xn.T * FP8_SCALE).astype(ml_dtypes.float8_e4m3)
    negeye = np.ascontiguousarray(
        (-2.0 * FP8_SCALE * FP8_SCALE) * np.eye(P, dtype=np.float16))
    in_maps = []
    for c in range(n_cores):
        s = c * BL
        rot = np.concatenate([xnt[:, s:], xnt[:, :s]], axis=1)[:, :CW]
        in_maps.append({"xt": np.ascontiguousarray(rot), "negeye": negeye})
    return in_maps


def postprocess(results, n_cores=N_CORES):
    """Stitch per-core row/col maxima and apply the scalar epilogue."""
    inv = 1.0 / (FP8_SCALE * FP8_SCALE)
    gmax = np.full(B, -np.inf, dtype=np.float64)
    for c in range(n_cores):
        s = c * BL
        rm = np.asarray(results[c]["rowmax"], dtype=np.float64)   # [P, MCH]
        # local row i = m*P + p  ->  global row s + i
        np.maximum(gmax[s:s + BL], rm.T.reshape(-1), out=gmax[s:s + BL])
        ca = np.asarray(results[c]["colacc"], dtype=np.float64)   # [P, CW]
        colmax = ca.max(axis=0)                                   # [CW]
        # rotated col j -> global row (s + j) mod B; CW < B so no dups
        n0 = min(CW, B - s)
        np.maximum(gmax[s:s + n0], colmax[:n0], out=gmax[s:s + n0])
        if n0 < CW:
            np.maximum(gmax[:CW - n0], colmax[n0:], out=gmax[:CW - n0])
    maxsim = gmax * inv
    d2 = 2.0 - 2.0 * maxsim + EPS
    loss = -0.5 * np.mean(np.log(d2))
    return np.array(loss, dtype=np.float32)


_NC_CACHE = {}


def _get_nc():
    key = (B, D, N_CORES)
    if key not in _NC_CACHE:
        _NC_CACHE[key] = build_nc()
    return _NC_CACHE[key]


def kernel(x, **_ignored):
    nc = _get_nc()
    in_maps = prepare_inputs(x)
    last_exc = None
    for _attempt in range(3):
        try:
            res = run_bass_kernel_spmd(nc, in_maps,
                                       core_ids=list(range(N_CORES)))
            return postprocess(res.results)
        except Exception as exc:  # transient NRT/tunnel hiccups
            last_exc = exc
    raise last_exc


if __name__ == "__main__":
    x = np.random.default_rng(0).standard_normal((B, D), dtype=np.float32)
    print(kernel(x))


# revision 9
# speedup vs baseline: 1.5692x; 1.1542x over previous
"""KoLeo loss kernel for 8 Trainium2 NeuronCores — symmetric-half version.

Reference computation (B=16384, D=1024):
    xn  = x / max(||x||_2, 1e-12)          # row L2-normalize
    sim = xn @ xn.T                        # B x B cosine similarity
    max_sim[i] = max_{j != i} sim[i, j]    # nearest neighbor (excl. self)
    out = -mean(log(sqrt(2 - 2*max_sim + 1e-8)))

Sharding + symmetry: rows of x are split across 8 cores (2048 rows each).
sim is symmetric, so each computed entry sim[i, j] can serve both row i's
max (row-max over the streamed block) and row j's max (column-max,
accumulated across row chunks).  Each 128-row chunk m of a core computes
only an 8320-column window starting at its own diagonal (rotated frame),
instead of the full 16384 columns.  8320 is the provable minimum for a
diagonal-anchored window: a pair {a, b} is missed by chunk(a) iff
(a%128 + delta) mod B in [C, B) and by chunk(b) iff delta in
(b%128, b%128 + B - C]; both fail only if 2C <= B + a%128 + b%128
<= 16638, so C = 8320 (2C = 16640) covers every pair.  This cuts the
matmul work to 0.508x of the full matrix (the true lower bound is 0.5).

Engine pipeline per psum block (width 512; a 128-wide tail per chunk):
    PE   : 4 fp8-DoubleRow matmuls  -> ps [128, w] f32 (PSUM) (853 ns)
    Act  : copy ps -> blk [128, w] fp16 (SBUF)                (~675 ns)
    DVE  : (t==0: blk[:, :128] += -2*scale^2*I)
           row-max: tensor_scalar(out=scratch, in0=blk,
               scalar1=-inf, op0=max, op1=max, accum_out=rowacc)
               — accum_out is the op1-max reduction of op0(in0, s1),
               i.e. an exact row-max of blk on the fast TensorScalar
               datapath (TensorReduce is capped at 1x = ~575 ns).
           acc[:, s:s+w] = max(blk, acc)   (column-max, tensor_max, 2x)
The fp16 bounce copy gives the DVE 2-byte SBUF operands (2x/4x tiers;
GPSIMD cannot run TensorTensor ops and cannot read PSUM on TRN2).
Row-max reduces only the fp16 block (never the cross-chunk
accumulator), so it is exact: the accumulator holds other rows' maxima
and must not leak into row-max.

Host finishes: per-column partition max of acc, scatter-max into the
global per-row max (the rotation makes that two slice maxes per core),
then the scalar log/sqrt/mean epilogue in float64.

Implementation notes:
  - Host pre-normalizes rows (f64) and pre-transposes to x.T in fp8e4m3
    scaled by 8 (DoubleRow perf mode, 2x matmul throughput; similarities
    come out scaled by 64, undone on the host).
  - Per-core input is x.T rotated so the core's own rows are columns
    0..2047; chunk m's window starts at column 128*m, so the
    self-similarity diagonal sits at window offset [0, 128) for every
    chunk — one -2*scale^2*I add per chunk kills the self-match.
  - Blocks are processed sorted by window-start column so the resident
    rhs DMA streams strictly left-to-right and finalized slices of the
    column-max accumulator DMA out while compute continues (no tail).
  - One resident SBUF tensor R [128, 8, 10240] (fp8) serves as both
    matmul weights (own rows = cols 0..2047) and moving data.  The
    input DMA uses 512-col pieces for the first 2048 columns (fast PE
    start) then 2048-col pieces (2 KB-per-partition descriptors keep
    the DMA queues efficient; 512-B descriptors measured 2.4x slower
    aggregate and stalled the PE ~58 us).
"""

import sys

if "/opt/trn_rl_repo" not in sys.path:
    sys.path.insert(0, "/opt/trn_rl_repo")

import os as _os

import numpy as np
import ml_dtypes

import concourse.bass as bass  # noqa: F401  (import keeps bass registered)
import concourse.mybir as mybir
import concourse.tile as tile
from concourse import bacc
from concourse.bass_utils import run_bass_kernel_spmd

P = 128          # SBUF partitions
NBLK = 512       # similarity column block width (= one PSUM bank of f32)
EPS = 1e-8

B = 16384        # rows of x
D = 1024         # feature dim
N_CORES = 8
BL = B // N_CORES          # local rows per core (2048)
MCH = BL // P              # row chunks per core (16)
WCOLS = 16 * NBLK + P      # window columns per chunk (8320, the minimum)
NT = 17                    # blocks per chunk window (16 full + 128 tail)
CW = (MCH - 1) * P + WCOLS  # resident rotated columns (10240)
KCH = D // P               # contraction chunks (8)
KSTEP = 2                  # fp8 DoubleRow: K chunks of 256

FP8_SCALE = 8.0
NEG_INIT = -60000.0        # fp16-representable, far below any -128..128 sim

# row-reduce flavor: "ts" = tensor_scalar w/ max-accum (fast path),
# "reduce" = plain reduce_max (1x, known-good fallback)
ROWRED = _os.environ.get("KOLEO_ROWRED", "ts")
MEMSET_ENG = _os.environ.get("KOLEO_MEMSET", "vector")  # "gpsimd" | "vector"

# input DMA piece boundaries (columns): fine first, then 2048-wide
_PIECES = [0, 512, 1024, 2048, 4096, 6144, 8192, CW]


def build_nc(n_cores=N_CORES):
    """Build the per-core SPMD Bass program.

    Inputs :  xt     [D, CW] fp8e4m3 — rotated, normalized, scaled x.T
              negeye [P, P]  f16 — the constant -2*scale^2 * I
    Outputs:  rowmax [P, MCH] f32 — rowmax[p, m] = scale^2 *
              max_{j in window} sim[128m+p, j] (excl. self)
              colacc [P, CW] f16 — colacc[p, c] = scale^2 *
              max over chunks m (with c in window m) of sim[128m+p, c]
    """
    in_dt = mybir.dt.float8e4
    f32 = mybir.dt.float32
    f16 = mybir.dt.float16
    perf_mode = mybir.MatmulPerfMode.DoubleRow

    nc = bacc.Bacc("TRN2", target_bir_lowering=False, debug=False,
                   num_devices=n_cores)
    xt = nc.dram_tensor("xt", [D, CW], in_dt, kind="ExternalInput")
    negeye = nc.dram_tensor("negeye", [P, P], f16, kind="ExternalInput")
    rowmax_out = nc.dram_tensor("rowmax", [P, MCH], f32,
                                kind="ExternalOutput")
    colacc_out = nc.dram_tensor("colacc", [P, CW], f16,
                                kind="ExternalOutput")
    xt_ap = xt[:]

    with tile.TileContext(nc) as tc:
        with (
            tc.tile_pool(name="data", bufs=1) as data_pool,
            tc.tile_pool(name="blk", bufs=8) as blk_pool,
            tc.tile_pool(name="psum", bufs=8, space="PSUM") as psum_pool,
            tc.tile_pool(name="stats", bufs=1) as stats_pool,
        ):
            R = data_pool.tile([P, KCH, CW], in_dt, name="R")
            acc = data_pool.tile([P, CW], f16, name="acc")
            eye = stats_pool.tile([P, P], f16, name="eye")
            scratch = stats_pool.tile([P, NBLK], f16, name="scratch")
            rowaccs = [
                stats_pool.tile([P, NT], f32, name=f"rowacc{m}",
                                tag=f"rowacc{m}")
                for m in range(MCH)
            ]
            rowmax = stats_pool.tile([P, MCH], f32, name="rowmax")

            # col-max accumulator starts far below any similarity
            memset_eng = nc.gpsimd if MEMSET_ENG == "gpsimd" else nc.vector
            memset_eng.memset(acc[:], NEG_INIT)
            nc.sync.dma_start(eye[:], negeye[:])

            # Stream the rotated slab left-to-right on the two HWDGE
            # queues (sync + scalar); gpsimd software DGE stays free.
            dma_eng = [nc.sync, nc.scalar]
            di = 0
            for j in range(len(_PIECES) - 1):
                c0, c1 = _PIECES[j], _PIECES[j + 1]
                for k in range(KCH):
                    dma_eng[di % 2].dma_start(
                        R[:, k, c0:c1], xt_ap[k * P:(k + 1) * P, c0:c1])
                    di += 1

            # All (chunk, block) pairs sorted by window-start column.
            # t < 16 are full 512-col blocks; t == 16 is the 128-col tail.
            blocks = sorted(
                (P * m + NBLK * t, m, t, NBLK if t < 16 else P)
                for m in range(MCH) for t in range(NT)
            )
            dma_ptr = 0
            for idx, (start, m, t, w) in enumerate(blocks):
                ps = psum_pool.tile([P, NBLK], f32, name="ps", tag="ps")
                for g in range(KCH // KSTEP):
                    k = g * KSTEP
                    lhsT = R[:, k:k + KSTEP, m * P:(m + 1) * P]
                    rhs = R[:, k:k + KSTEP, start:start + w]
                    nc.tensor.matmul(
                        ps[:, 0:w], lhsT, rhs,
                        start=(g == 0),
                        stop=(k + KSTEP == KCH),
                        perf_mode=perf_mode,
                    )
                # bounce PSUM f32 -> SBUF fp16 on the Act engine
                blk = blk_pool.tile([P, NBLK], f16, name="blk", tag="blk")
                nc.scalar.copy(blk[:, 0:w], ps[:, 0:w])
                if t == 0:
                    # self-similarity lives at blk[p, p]: add -2*scale^2*I
                    nc.vector.tensor_add(
                        out=blk[:, 0:P], in0=blk[:, 0:P], in1=eye[:])
                # exact row-max of this block only (never the accumulator)
                if ROWRED == "ts":
                    nc.vector.tensor_scalar(
                        out=scratch[:, 0:w],
                        in0=blk[:, 0:w],
                        scalar1=NEG_INIT,
                        scalar2=None,
                        op0=mybir.AluOpType.max,
                        op1=mybir.AluOpType.max,
                        accum_out=rowaccs[m][:, t:t + 1],
                    )
                else:
                    nc.vector.reduce_max(
                        out=rowaccs[m][:, t:t + 1],
                        in_=blk[:, 0:w],
                        axis=mybir.AxisListType.X,
                        op=mybir.AluOpType.max,
                    )
                # column-max accumulate (fp16 SBUF -> 2x DVE tier)
                nc.vector.tensor_max(
                    out=acc[:, start:start + w],
                    in0=blk[:, 0:w],
                    in1=acc[:, start:start + w],
                )
                # Everything left of the next block's start is final —
                # stream it out while compute continues.
                next_start = CW if idx == len(blocks) - 1 else blocks[idx + 1][0]
                if next_start - dma_ptr >= 2048 or idx == len(blocks) - 1:
                    nc.sync.dma_start(
                        colacc_out[:, dma_ptr:next_start],
                        acc[:, dma_ptr:next_start])
                    dma_ptr = next_start

            for m in range(MCH):
                nc.vector.reduce_max(
                    out=rowmax[:, m:m + 1],
                    in_=rowaccs[m][:],
                    axis=mybir.AxisListType.X,
                    op=mybir.AluOpType.max,
                )
            nc.sync.dma_start(rowmax_out[:], rowmax[:])

    nc.compile()
    return nc


def prepare_inputs(x, n_cores=N_CORES):
    """Host prep: normalize (f64), transpose, scale+cast fp8, rotate."""
    xd = np.asarray(x, dtype=np.float64)
    norms = np.sqrt(np.einsum("ij,ij->i", xd, xd))
    np.maximum(norms, 1e-12, out=norms)
    xn = xd / norms[:, None]
    xnt = np.ascontiguousarray(xn.T * FP8_SCALE).astype(ml_dtypes.float8_e4m3)
    negeye = np.ascontiguousarray(
        (-2.0 * FP8_SCALE * FP8_SCALE) * np.eye(P, dtype=np.float16))
    in_maps = []
    for c in range(n_cores):
        s = c * BL
        rot = np.concatenate([xnt[:, s:], xnt[:, :s]], axis=1)[:, :CW]
        in_maps.append({"xt": np.ascontiguousarray(rot), "negeye": negeye})
    return in_maps


def postprocess(results, n_cores=N_CORES):
    """Stitch per-core row/col maxima and apply the scalar epilogue."""
    inv = 1.0 / (FP8_SCALE * FP8_SCALE)
    gmax = np.full(B, -np.inf, dtype=np.float64)
    for c in range(n_cores):
        s = c * BL
        rm = np.asarray(results[c]["rowmax"], dtype=np.float64)   # [P, MCH]
        # local row i = m*P + p  ->  global row s + i
        np.maximum(gmax[s:s + BL], rm.T.reshape(-1), out=gmax[s:s + BL])
        ca = np.asarray(results[c]["colacc"], dtype=np.float64)   # [P, CW]
        colmax = ca.max(axis=0)                                   # [CW]
        # rotated col j -> global row (s + j) mod B; CW < B so no dups
        n0 = min(CW, B - s)
        np.maximum(gmax[s:s + n0], colmax[:n0], out=gmax[s:s + n0])
        if n0 < CW:
            np.maximum(gmax[:CW - n0], colmax[n0:], out=gmax[:CW - n0])
    maxsim = gmax * inv
    d2 = 2.0 - 2.0 * maxsim + EPS
    loss = -0.5 * np.mean(np.log(d2))
    return np.array(loss, dtype=np.float32)


_NC_CACHE = {}


def _get_nc():
    key = (B, D, N_CORES)
    if key not in _NC_CACHE:
        _NC_CACHE[key] = build_nc()
    return _NC_CACHE[key]


def kernel(x, **_ignored):
    nc = _get_nc()
    in_maps = prepare_inputs(x)
    last_exc = None
    for _attempt in range(3):
        try:
            res = run_bass_kernel_spmd(nc, in_maps,
                                       core_ids=list(range(N_CORES)))
            return postprocess(res.results)
        except Exception as exc:  # transient NRT/tunnel hiccups
            last_exc = exc
    raise last_exc


if __name__ == "__main__":
    x = np.random.default_rng(0).standard_normal((B, D), dtype=np.float32)
    print(kernel(x))


# revision 11
# speedup vs baseline: 1.7146x; 1.0927x over previous
"""KoLeo loss kernel for 8 Trainium2 NeuronCores — symmetric-half version.

Reference computation (B=16384, D=1024):
    xn  = x / max(||x||_2, 1e-12)          # row L2-normalize
    sim = xn @ xn.T                        # B x B cosine similarity
    max_sim[i] = max_{j != i} sim[i, j]    # nearest neighbor (excl. self)
    out = -mean(log(sqrt(2 - 2*max_sim + 1e-8)))

Sharding + symmetry: rows of x are split across 8 cores (2048 rows each).
sim is symmetric, so each computed entry sim[i, j] can serve both row i's
max (row-max over the streamed block) and row j's max (column-max,
accumulated across row chunks).  Each 128-row chunk m of a core computes
only an 8320-column window starting at its own diagonal (rotated frame),
instead of the full 16384 columns.  8320 is the provable minimum for a
diagonal-anchored window: a pair {a, b} is missed by chunk(a) iff
(a%128 + delta) mod B in [C, B) and by chunk(b) iff delta in
(b%128, b%128 + B - C]; both fail only if 2C <= B + a%128 + b%128
<= 16638, so C = 8320 (2C = 16640) covers every pair.  This cuts the
matmul work to 0.508x of the full matrix (the true lower bound is 0.5).

Engine pipeline per psum block (width 512; a 128-wide tail per chunk):
    PE   : 4 fp8-DoubleRow matmuls  -> ps [128, w] f32 (PSUM) (853 ns)
    Act  : copy ps -> blk [128, w] fp16 (SBUF)                (~675 ns)
    DVE  : (t==0: blk[:, :128] += -2*scale^2*I)
           rowbuf_m = max(rowbuf_m, blk)   (row-candidate accumulate)
           acc[:, s:s+w] = max(blk, acc)   (column-max)
Both DVE ops are elementwise tensor_max (TensorTensor 2x tier on fp16
SBUF, ~325 ns) — reduce-class DVE ops are capped at 1x (~575 ns), so
the row direction also accumulates elementwise into a per-chunk
[128, 512] buffer and is reduced ONCE per chunk at the end.
The fp16 bounce copy gives the DVE 2-byte SBUF operands (2x/4x tiers;
GPSIMD cannot run TensorTensor ops and cannot read PSUM on TRN2).
Row-max reduces only the fp16 block (never the cross-chunk
accumulator), so it is exact: the accumulator holds other rows' maxima
and must not leak into row-max.

Host finishes: per-column partition max of acc, scatter-max into the
global per-row max (the rotation makes that two slice maxes per core),
then the scalar log/sqrt/mean epilogue in float64.

Implementation notes:
  - Host pre-normalizes rows (f64) and pre-transposes to x.T in fp8e4m3
    scaled by 8 (DoubleRow perf mode, 2x matmul throughput; similarities
    come out scaled by 64, undone on the host).
  - Per-core input is x.T rotated so the core's own rows are columns
    0..2047; chunk m's window starts at column 128*m, so the
    self-similarity diagonal sits at window offset [0, 128) for every
    chunk — one -2*scale^2*I add per chunk kills the self-match.
  - Blocks are processed sorted by window-start column so the resident
    rhs DMA streams strictly left-to-right and finalized slices of the
    column-max accumulator DMA out while compute continues (no tail).
  - One resident SBUF tensor R [128, 8, 10240] (fp8) serves as both
    matmul weights (own rows = cols 0..2047) and moving data.  The
    input DMA uses 512-col pieces for the first 2048 columns (fast PE
    start) then 2048-col pieces (2 KB-per-partition descriptors keep
    the DMA queues efficient; 512-B descriptors measured 2.4x slower
    aggregate and stalled the PE ~58 us).
"""

import sys

if "/opt/trn_rl_repo" not in sys.path:
    sys.path.insert(0, "/opt/trn_rl_repo")

import os as _os

import numpy as np
import ml_dtypes

import concourse.bass as bass  # noqa: F401  (import keeps bass registered)
import concourse.mybir as mybir
import concourse.tile as tile
from concourse import bacc
from concourse.bass_utils import run_bass_kernel_spmd

P = 128          # SBUF partitions
NBLK = 512       # similarity column block width (= one PSUM bank of f32)
EPS = 1e-8

B = 16384        # rows of x
D = 1024         # feature dim
N_CORES = 8
BL = B // N_CORES          # local rows per core (2048)
MCH = BL // P              # row chunks per core (16)
WCOLS = 16 * NBLK + P      # window columns per chunk (8320, the minimum)
NT = 17                    # blocks per chunk window (16 full + 128 tail)
CW = (MCH - 1) * P + WCOLS  # resident rotated columns (10240)
KCH = D // P               # contraction chunks (8)
KSTEP = 2                  # fp8 DoubleRow: K chunks of 256

FP8_SCALE = 8.0
NEG_INIT = -60000.0        # fp16-representable, far below any -128..128 sim

# row-reduce flavor: "ts" = tensor_scalar w/ max-accum (fast path),
# "reduce" = plain reduce_max (1x, known-good fallback)
ROWRED = _os.environ.get("KOLEO_ROWRED", "ts")
MEMSET_ENG = _os.environ.get("KOLEO_MEMSET", "vector")  # "gpsimd" | "vector"

# input DMA piece boundaries (columns): fine first, then wide
_PIECES = [0, 512, 1536, 3584, 5632, 7680, CW]


def build_nc(n_cores=N_CORES):
    """Build the per-core SPMD Bass program.

    Inputs :  xt     [D, CW] fp8e4m3 — rotated, normalized, scaled x.T
              negeye [P, P]  f16 — the constant -2*scale^2 * I
    Outputs:  rowmax [P, MCH] f32 — rowmax[p, m] = scale^2 *
              max_{j in window} sim[128m+p, j] (excl. self)
              colacc [P, CW] f16 — colacc[p, c] = scale^2 *
              max over chunks m (with c in window m) of sim[128m+p, c]
    """
    in_dt = mybir.dt.float8e4
    f32 = mybir.dt.float32
    f16 = mybir.dt.float16
    perf_mode = mybir.MatmulPerfMode.DoubleRow

    nc = bacc.Bacc("TRN2", target_bir_lowering=False, debug=False,
                   num_devices=n_cores)
    xt = nc.dram_tensor("xt", [D, CW], in_dt, kind="ExternalInput")
    negeye = nc.dram_tensor("negeye", [P, P], f16, kind="ExternalInput")
    rowmax_out = nc.dram_tensor("rowmax", [P, MCH], f32,
                                kind="ExternalOutput")
    colacc_out = nc.dram_tensor("colacc", [P, CW], f16,
                                kind="ExternalOutput")
    xt_ap = xt[:]

    with tile.TileContext(nc) as tc:
        with (
            tc.tile_pool(name="data", bufs=1) as data_pool,
            tc.tile_pool(name="blk", bufs=8) as blk_pool,
            tc.tile_pool(name="psum", bufs=8, space="PSUM") as psum_pool,
            tc.tile_pool(name="stats", bufs=1) as stats_pool,
        ):
            R = data_pool.tile([P, KCH, CW], in_dt, name="R")
            acc = data_pool.tile([P, CW], f16, name="acc")
            eye = stats_pool.tile([P, P], f16, name="eye")
            rowbufs = [
                stats_pool.tile([P, NBLK], f16, name=f"rowbuf{m}",
                                tag=f"rowbuf{m}")
                for m in range(MCH)
            ]
            rowmax = stats_pool.tile([P, MCH], f32, name="rowmax")

            # col-max accumulator starts far below any similarity
            memset_eng = nc.gpsimd if MEMSET_ENG == "gpsimd" else nc.vector
            memset_eng.memset(acc[:], NEG_INIT)
            nc.sync.dma_start(eye[:], negeye[:])

            # Stream the rotated slab left-to-right on the two HWDGE
            # queues (sync + scalar); gpsimd software DGE stays free.
            dma_eng = [nc.sync, nc.scalar]
            di = 0
            for j in range(len(_PIECES) - 1):
                c0, c1 = _PIECES[j], _PIECES[j + 1]
                for k in range(KCH):
                    dma_eng[di % 2].dma_start(
                        R[:, k, c0:c1], xt_ap[k * P:(k + 1) * P, c0:c1])
                    di += 1

            # All (chunk, block) pairs sorted by window-start column.
            # t < 16 are full 512-col blocks; t == 16 is the 128-col tail.
            blocks = sorted(
                (P * m + NBLK * t, m, t, NBLK if t < 16 else P)
                for m in range(MCH) for t in range(NT)
            )
            dma_ptr = 0
            for idx, (start, m, t, w) in enumerate(blocks):
                ps = psum_pool.tile([P, NBLK], f32, name="ps", tag="ps")
                for g in range(KCH // KSTEP):
                    k = g * KSTEP
                    lhsT = R[:, k:k + KSTEP, m * P:(m + 1) * P]
                    rhs = R[:, k:k + KSTEP, start:start + w]
                    nc.tensor.matmul(
                        ps[:, 0:w], lhsT, rhs,
                        start=(g == 0),
                        stop=(k + KSTEP == KCH),
                        perf_mode=perf_mode,
                    )
                # bounce PSUM f32 -> SBUF fp16 on the Act engine
                blk = blk_pool.tile([P, NBLK], f16, name="blk", tag="blk")
                nc.scalar.copy(blk[:, 0:w], ps[:, 0:w])
                if t == 0:
                    # self-similarity lives at blk[p, p]: add -2*scale^2*I
                    nc.vector.tensor_add(
                        out=blk[:, 0:P], in0=blk[:, 0:P], in1=eye[:])
                # row-candidate accumulate: rowbuf_m = max(rowbuf_m, blk)
                # elementwise over the 512 lane-columns (TensorTensor 2x
                # tier, ~325 ns) — reduce-class ops are 1x-capped (~575
                # ns) so the per-chunk reduction happens ONCE at the end.
                # t == 0 seeds rowbuf with a copy (4x tier), no memset.
                if t == 0:
                    nc.vector.tensor_copy(
                        out=rowbufs[m][:], in_=blk[:])
                else:
                    nc.vector.tensor_max(
                        out=rowbufs[m][:, 0:w],
                        in0=blk[:, 0:w],
                        in1=rowbufs[m][:, 0:w],
                    )
                # column-max accumulate (fp16 SBUF -> 2x DVE tier)
                nc.vector.tensor_max(
                    out=acc[:, start:start + w],
                    in0=blk[:, 0:w],
                    in1=acc[:, start:start + w],
                )
                # Everything left of the next block's start is final —
                # stream it out while compute continues.
                next_start = CW if idx == len(blocks) - 1 else blocks[idx + 1][0]
                if next_start - dma_ptr >= 2048 or idx == len(blocks) - 1:
                    nc.sync.dma_start(
                        colacc_out[:, dma_ptr:next_start],
                        acc[:, dma_ptr:next_start])
                    dma_ptr = next_start

            for m in range(MCH):
                nc.vector.reduce_max(
                    out=rowmax[:, m:m + 1],
                    in_=rowbufs[m][:],
                    axis=mybir.AxisListType.X,
                    op=mybir.AluOpType.max,
                )
            nc.sync.dma_start(rowmax_out[:], rowmax[:])

    nc.compile()
    return nc


def prepare_inputs(x, n_cores=N_CORES):
    """Host prep: normalize (f64), transpose, scale+cast fp8, rotate."""
    xd = np.asarray(x, dtype=np.float64)
    norms = np.sqrt(np.einsum("ij,ij->i", xd, xd))
    np.maximum(norms, 1e-12, out=norms)
    xn = xd / norms[:, None]
    xnt = np.ascontiguousarray(xn.T * FP8_SCALE).astype(ml_dtypes.float8_e4m3)
    negeye = np.ascontiguousarray(
        (-2.0 * FP8_SCALE * FP8_SCALE) * np.eye(P, dtype=np.float16))
    in_maps = []
    for c in range(n_cores):
        s = c * BL
        rot = np.concatenate([xnt[:, s:], xnt[:, :s]], axis=1)[:, :CW]
        in_maps.append({"xt": np.ascontiguousarray(rot), "negeye": negeye})
    return in_maps


def postprocess(results, n_cores=N_CORES):
    """Stitch per-core row/col maxima and apply the scalar epilogue."""
    inv = 1.0 / (FP8_SCALE * FP8_SCALE)
    gmax = np.full(B, -np.inf, dtype=np.float64)
    for c in range(n_cores):
        s = c * BL
        rm = np.asarray(results[c]["rowmax"], dtype=np.float64)   # [P, MCH]
        # local row i = m*P + p  ->  global row s + i
        np.maximum(gmax[s:s + BL], rm.T.reshape(-1), out=gmax[s:s + BL])
        ca = np.asarray(results[c]["colacc"], dtype=np.float64)   # [P, CW]
        colmax = ca.max(axis=0)                                   # [CW]
        # rotated col j -> global row (s + j) mod B; CW < B so no dups
        n0 = min(CW, B - s)
        np.maximum(gmax[s:s + n0], colmax[:n0], out=gmax[s:s + n0])
        if n0 < CW:
            np.maximum(gmax[:CW - n0], colmax[n0:], out=gmax[:CW - n0])
    maxsim = gmax * inv
    d2 = 2.0 - 2.0 * maxsim + EPS
    loss = -0.5 * np.mean(np.log(d2))
    return np.array(loss, dtype=np.float32)


_NC_CACHE = {}


def _get_nc():
    key = (B, D, N_CORES)
    if key not in _NC_CACHE:
        _NC_CACHE[key] = build_nc()
    return _NC_CACHE[key]


def kernel(x, **_ignored):
    nc = _get_nc()
    in_maps = prepare_inputs(x)
    last_exc = None
    for _attempt in range(3):
        try:
            res = run_bass_kernel_spmd(nc, in_maps,
                                       core_ids=list(range(N_CORES)))
            return postprocess(res.results)
        except Exception as exc:  # transient NRT/tunnel hiccups
            last_exc = exc
    raise last_exc


if __name__ == "__main__":
    x = np.random.default_rng(0).standard_normal((B, D), dtype=np.float32)
    print(kernel(x))
